# revision 5
# baseline (speedup 1.0000x reference)
"""Trainium2 Bass kernel for nn_DeconvLayer (cascaded order-16 IIR along rows).

v3: wide XBAR DMA-transpose edition.

The cascaded recurrence is a truncated FIR y = g (*) x (columns 0..15 of x
zeroed), realized as Toeplitz matmuls with time on SBUF partitions.

Transposed input tiles are loaded straight from DRAM with DmaTransposeAnt.
The cost model runs each DMA's completion 1716 ns after its engine slice and
round-robins completions over 8 semaphore lanes, so many small DMAs throttle
on lane reuse (a 112 ns 128x128 transpose still occupies its lane ~1.8 us).
v3 therefore transposes [128, 1024] blocks (one DMA = 8 time tiles, 896 ns
engine / ~2.6 us lane): 16 loads instead of 128.

Engine balance (greedy over cost-model ns): SP+ACT stream the transposed
loads (HWDGE-only op) and some stores, DVE+Pool split the PSUM->SBUF fp16
cast-copies, stores go to the least-loaded of SP/ACT/Pool. A short PE
warm-up (zero matmuls into a scratch PSUM bank) starts the p-state ramp
clock so real conv matmuls run at the full 2.4 GHz almost immediately.
"""

import os
import time

import numpy as np

# the trace path needs antenv.axon_hooks, absent in this container; make
# sure a stray BASS_TRACE in the caller's env can't break execution
os.environ.setdefault("BASS_NEVER_TRACE", "1")

import concourse.bass as bass
import concourse.mybir as mybir
from concourse.bass_utils import run_bass_kernel_spmd
from concourse.tile import TileContext

N_CORES = 8
ROWS = 4096
COLS = 4096
ROWS_PER_CORE = ROWS // N_CORES  # 512
K_TAPS = 16
T_FIR = 256
NRC = ROWS_PER_CORE // 128  # 4 row chunks per core
NBLK = COLS // 2048         # 2 transposed 2048-col blocks per row chunk

_F16 = mybir.dt.float16
_F32 = mybir.dt.float32

# --- tuning knobs ---------------------------------------------------------
P_S = 32        # deep-matmul width / min tap coverage-1 (rms err ~1.8e-3)
P_PO = 3        # conv-PSUM tiles of 2 banks each (+1 warmup bank)
P_Y = 8         # output-group SBUF slots (no reuse back-pressure on copies)
P_U = 4         # buffers per transposed-block tag (4KB/part each)
N_WARM = 6      # PE warm-up matmuls (512 cols each)

# cost-model ns estimates used for static engine balancing
_C_TPOSE = 1792.0  # DmaTransposeAnt [128, 2048]
_C_STORE = 1579.0  # [128, 2048] fp16 store
_C_COPY = {"DVE": 1192.0, "ACT": 1120.0}


def _impulse_response(h: np.ndarray, n: int) -> np.ndarray:
    """Impulse response of v[i] = x[i] + sum_j h[j] v[i-1-j], float64."""
    g = np.zeros(n, np.float64)
    g[0] = 1.0
    K = len(h)
    for t in range(1, n):
        lo = max(0, t - K)
        g[t] = np.dot(h[: t - lo], g[t - 1 : lo - 1 if lo > 0 else None : -1])
    return g


def _build_g_cat(h32: np.ndarray) -> np.ndarray:
    """[128, 128 + P_S] fp16 Toeplitz slabs [G_loc | G_deep].

    G_loc[k, m]  = g[m - k]        (own-tile taps [0, m], all 128 cols)
    G_deep[k, m] = g[128 + m - k]  (prev-tile taps [m+1, m+128]; only the
                                    first P_S output cols get the deep term)
    """
    S = P_S
    h = h32.astype(np.float64)
    g1 = _impulse_response(h, T_FIR)
    g2 = _impulse_response(h[::-1], T_FIR)
    gc = np.convolve(g1, g2)[:T_FIR]
    kk = np.arange(128)[:, None]

    def toe(offs, width):
        mm = np.arange(width)[None, :]
        t = offs + mm - kk
        valid = (t >= 0) & (t < T_FIR)
        return np.where(valid, gc[np.clip(t, 0, T_FIR - 1)], 0.0)

    out = np.concatenate([toe(0, 128), toe(128, S)], axis=1)
    return out.astype(np.float16)


class _Greedy:
    """Static engine balancer over cost-model ns estimates."""

    def __init__(self, nc):
        self.eng = {
            "SP": nc.sync,
            "ACT": nc.scalar,
            "Pool": nc.gpsimd,
            "DVE": nc.vector,
        }
        self.busy = {k: 0.0 for k in self.eng}

    def pick(self, names, cost):
        k = min(names, key=lambda n: self.busy[n] + (cost[n] if isinstance(cost, dict) else cost))
        self.busy[k] += cost[k] if isinstance(cost, dict) else cost
        return k, self.eng[k]


def _build_program(legalize: bool = True) -> bass.Bass:
    nc = bass.Bass()
    x = nc.dram_tensor("x", [ROWS_PER_CORE, COLS], _F16, kind="ExternalInput")
    g = nc.dram_tensor("g", [128, 128 + P_S], _F16, kind="ExternalInput")
    y = nc.dram_tensor("y", [ROWS_PER_CORE, COLS], _F16, kind="ExternalOutput")

    S = P_S
    gb = _Greedy(nc)
    # ACT leads: its first HWDGE DMA issues without the cross-ring
    # serializing wait the sem-assignment pass pins on the other ring's
    # opener, so gt (needed by the first conv matmul) lands earliest
    spa = [nc.scalar, nc.sync, nc.sync, nc.scalar, nc.sync, nc.sync, nc.scalar, nc.sync]
    tp_i = 0

    with TileContext(nc) as tc:
        with (
            tc.tile_pool(name="cpool", bufs=1) as cpool,
            tc.tile_pool(name="upool", bufs=P_U) as upool,
            tc.tile_pool(name="pwpool", bufs=1, space="PSUM") as pwpool,
            tc.tile_pool(name="popool", bufs=P_PO, space="PSUM") as popool,
            tc.tile_pool(name="ypool", bufs=P_Y) as ypool,
        ):
            # PE warm-up: zero matmuls into a scratch PSUM bank start the
            # p-state ramp clock while the first transposed loads are in
            # flight (their ~2.6us DGE+sem latency would otherwise be spent
            # ramping through the slow p-states on real work)
            zt = cpool.tile([128, 512], _F16, tag="z")
            nc.vector.memset(zt[:], 0.0)
            pw = pwpool.tile([128, 512], _F32, tag="pw")
            for _ in range(N_WARM):
                nc.tensor.matmul(
                    pw[:], lhsT=zt[:, 0:128], rhs=zt[:], start=True, stop=True
                )

            gt = cpool.tile([128, 128 + S], _F16, tag="g")
            nc.scalar.dma_start(gt[:], g[:])
            gb.busy["ACT"] += 500.0

            for rc in range(NRC):
                rs = slice(128 * rc, 128 * (rc + 1))

                # 2 transposed block loads per rc, issued up front:
                # ub[t, 128j + r] = x[rs, 2048*blk + 128j + t][r]
                u_blocks = {}
                for blk in range(NBLK):
                    ut = upool.tile([128, 2048], _F16, tag=f"u{blk}")
                    eng = spa[tp_i % 8]
                    tp_i += 1
                    eng.dma_start_transpose(
                        ut[:].rearrange("p (j r) -> p j r", j=16),
                        x[rs, 2048 * blk : 2048 * (blk + 1)],
                    )
                    gb.busy["SP" if eng is nc.sync else "ACT"] += _C_TPOSE
                    u_blocks[blk] = ut

                def u_tile(b, u_blocks=u_blocks):
                    # time tile b (0..31) -> 128-col slice of its block
                    return u_blocks[b // 16][:, 128 * (b % 16) : 128 * (b % 16 + 1)]

                for grp in range(4):  # 1024-col conv groups (2-bank PSUM)
                    pt = popool.tile([128, 1024], _F32, tag="po")
                    for h in range(2):  # 512-col halves, one PSUM bank each
                        q = 2 * grp + h
                        plan = []
                        for j in range(4):
                            plan.append((128 * j, 128, 4 * q + j, 0))
                        for j in range(4):
                            if 4 * q + j - 1 >= 0:
                                plan.append((128 * j, S, 4 * q + j - 1, 128))
                        for i, (col, w, b, goff) in enumerate(plan):
                            nc.tensor.matmul(
                                pt[:, 512 * h + col : 512 * h + col + w],
                                lhsT=u_tile(b),
                                rhs=gt[:, goff : goff + w],
                                start=(i == 0),
                                stop=(i == len(plan) - 1),
                            )

                    # y staging: 2048-col fp16 groups, cast-copied in
                    # 1024-col units split DVE/ACT (Pool's gpsimd cannot
                    # read PSUM on hardware)
                    q = 2 * grp + 1
                    if grp % 2 == 0:
                        yt = ypool.tile([128, 2048], _F16, tag="y")
                    _, eng = gb.pick(("DVE", "ACT"), _C_COPY)
                    cfn = eng.copy if eng is nc.scalar else eng.tensor_copy
                    cfn(yt[:, 1024 * (grp % 2) : 1024 * (grp % 2 + 1)], pt[:])
                    if q % 4 == 3:
                        c0 = 2048 * (q // 4)
                        # store schedule: early stores stay on the SP/ACT
                        # HWDGE ring (densifying it keeps the sem-merge
                        # pass's lane waits pointing at nearby DMAs instead
                        # of the final transpose); Pool takes rc2, and the
                        # tail is split 1024-wide across all three engines
                        if rc == NRC - 1:
                            for hh in range(2):
                                _, eng = gb.pick(("SP", "ACT", "Pool"), _C_STORE / 2)
                                eng.dma_start(
                                    y[rs, c0 + 1024 * hh : c0 + 1024 * (hh + 1)],
                                    yt[:, 1024 * hh : 1024 * (hh + 1)],
                                )
                        elif rc == 2:
                            gb.busy["Pool"] += _C_STORE
                            nc.gpsimd.dma_start(y[rs, c0 : c0 + 2048], yt[:])
                        else:
                            _, eng = gb.pick(("SP", "ACT"), _C_STORE)
                            eng.dma_start(y[rs, c0 : c0 + 2048], yt[:])
    if legalize:
        _legalize_waits(nc)
    return nc


def _legalize_waits(nc: bass.Bass) -> None:
    """This toolchain's walrus accepts at most ONE semaphore wait per
    instruction (Drain/EventSemaphore excepted), but Tile's semaphore
    assignment freely emits 2-3. Hoist extra waits onto injected same-engine
    NoOps placed immediately before the instruction — engines execute their
    stream serially (and a DMA trigger precedes its descriptor execution),
    so waiting earlier on the same engine preserves semantics.
    """
    for fn in nc.m.functions:
        for blk in fn.blocks:
            out = []
            changed = False
            for i in blk.instructions:
                tn = type(i).__name__
                si = i.sync_info
                cap = 2 if tn == "InstEventSemaphore" else 1
                if si is not None and len(si.on_wait) > cap:
                    waits = list(si.on_wait)
                    for w in waits[:-cap]:
                        out.append(
                            mybir.InstNoOp(
                                name=nc.get_next_instruction_name(),
                                ins=[],
                                outs=[],
                                engine=i.engine,
                                sync_info=mybir.SyncInfo(
                                    on_wait=[w], on_update=[]
                                ),
                            )
                        )
                    i.sync_info = mybir.SyncInfo(
                        on_wait=waits[-cap:], on_update=list(si.on_update)
                    )
                    changed = True
                out.append(i)
            if changed:
                blk.instructions = out


_PROGRAM = None


def kernel(**inputs: np.ndarray) -> np.ndarray:
    global _PROGRAM
    x = np.asarray(inputs["inputs"], dtype=np.float32)
    h = np.asarray(inputs["kernel"], dtype=np.float32)[0]
    assert x.shape == (ROWS, COLS) and h.shape == (K_TAPS,)

    g_cat = _build_g_cat(h)
    xm = x.astype(np.float16)
    xm[:, :K_TAPS] = 0

    if _PROGRAM is None:
        _PROGRAM = _build_program()

    in_maps = [
        {
            "x": xm[ROWS_PER_CORE * c : ROWS_PER_CORE * (c + 1)],
            "g": g_cat,
        }
        for c in range(N_CORES)
    ]
    # the axon-proxied device occasionally reports a transient
    # NRT_EXEC_UNIT_UNRECOVERABLE; a retry succeeds
    last_err = None
    for _ in range(3):
        try:
            res = run_bass_kernel_spmd(
                _PROGRAM, in_maps, list(range(N_CORES))
            ).results
            break
        except Exception as e:  # noqa: BLE001
            last_err = e
            time.sleep(2.0)
    else:
        raise last_err
    out = np.concatenate([res[c]["y"] for c in range(N_CORES)], axis=0)
    return out.astype(np.float32)


# revision 6
# speedup vs baseline: 1.0129x; 1.0129x over previous
"""Trainium2 Bass kernel for nn_DeconvLayer (cascaded order-16 IIR along rows).

v3: wide XBAR DMA-transpose edition.

The cascaded recurrence is a truncated FIR y = g (*) x (columns 0..15 of x
zeroed), realized as Toeplitz matmuls with time on SBUF partitions.

Transposed input tiles are loaded straight from DRAM with DmaTransposeAnt.
The cost model runs each DMA's completion 1716 ns after its engine slice and
round-robins completions over 8 semaphore lanes, so many small DMAs throttle
on lane reuse (a 112 ns 128x128 transpose still occupies its lane ~1.8 us).
v3 therefore transposes [128, 1024] blocks (one DMA = 8 time tiles, 896 ns
engine / ~2.6 us lane): 16 loads instead of 128.

Engine balance (greedy over cost-model ns): SP+ACT stream the transposed
loads (HWDGE-only op) and some stores, DVE+Pool split the PSUM->SBUF fp16
cast-copies, stores go to the least-loaded of SP/ACT/Pool. A short PE
warm-up (zero matmuls into a scratch PSUM bank) starts the p-state ramp
clock so real conv matmuls run at the full 2.4 GHz almost immediately.
"""

import os
import time

import numpy as np

# the trace path needs antenv.axon_hooks, absent in this container; make
# sure a stray BASS_TRACE in the caller's env can't break execution
os.environ.setdefault("BASS_NEVER_TRACE", "1")

import concourse.bass as bass
import concourse.mybir as mybir
from concourse.bass_utils import run_bass_kernel_spmd
from concourse.tile import TileContext

N_CORES = 8
ROWS = 4096
COLS = 4096
ROWS_PER_CORE = ROWS // N_CORES  # 512
K_TAPS = 16
T_FIR = 256
NRC = ROWS_PER_CORE // 128  # 4 row chunks per core
NBLK = COLS // 2048         # 2 transposed 2048-col blocks per row chunk

_F16 = mybir.dt.float16
_F32 = mybir.dt.float32

# --- tuning knobs ---------------------------------------------------------
P_S = 32        # deep-matmul width / min tap coverage-1 (rms err ~1.8e-3)
P_PO = 3        # conv-PSUM tiles of 2 banks each (+1 warmup bank)
P_Y = 8         # output-group SBUF slots (no reuse back-pressure on copies)
P_U = 4         # buffers per transposed-block tag (4KB/part each)
N_WARM = 6      # PE warm-up matmuls (512 cols each)

# cost-model ns estimates used for static engine balancing
_C_TPOSE = 1792.0  # DmaTransposeAnt [128, 2048]
_C_STORE = 1579.0  # [128, 2048] fp16 store
_C_COPY = {"DVE": 1192.0, "ACT": 1120.0}


def _impulse_response(h: np.ndarray, n: int) -> np.ndarray:
    """Impulse response of v[i] = x[i] + sum_j h[j] v[i-1-j], float64."""
    g = np.zeros(n, np.float64)
    g[0] = 1.0
    K = len(h)
    for t in range(1, n):
        lo = max(0, t - K)
        g[t] = np.dot(h[: t - lo], g[t - 1 : lo - 1 if lo > 0 else None : -1])
    return g


def _build_g_cat(h32: np.ndarray) -> np.ndarray:
    """[128, 128 + P_S] fp16 Toeplitz slabs [G_loc | G_deep].

    G_loc[k, m]  = g[m - k]        (own-tile taps [0, m], all 128 cols)
    G_deep[k, m] = g[128 + m - k]  (prev-tile taps [m+1, m+128]; only the
                                    first P_S output cols get the deep term)
    """
    S = P_S
    h = h32.astype(np.float64)
    g1 = _impulse_response(h, T_FIR)
    g2 = _impulse_response(h[::-1], T_FIR)
    gc = np.convolve(g1, g2)[:T_FIR]
    kk = np.arange(128)[:, None]

    def toe(offs, width):
        mm = np.arange(width)[None, :]
        t = offs + mm - kk
        valid = (t >= 0) & (t < T_FIR)
        return np.where(valid, gc[np.clip(t, 0, T_FIR - 1)], 0.0)

    out = np.concatenate([toe(0, 128), toe(128, S)], axis=1)
    return out.astype(np.float16)


class _Greedy:
    """Static engine balancer over cost-model ns estimates."""

    def __init__(self, nc):
        self.eng = {
            "SP": nc.sync,
            "ACT": nc.scalar,
            "Pool": nc.gpsimd,
            "DVE": nc.vector,
        }
        self.busy = {k: 0.0 for k in self.eng}

    def pick(self, names, cost):
        k = min(names, key=lambda n: self.busy[n] + (cost[n] if isinstance(cost, dict) else cost))
        self.busy[k] += cost[k] if isinstance(cost, dict) else cost
        return k, self.eng[k]


def _build_program(legalize: bool = True) -> bass.Bass:
    nc = bass.Bass()
    x = nc.dram_tensor("x", [ROWS_PER_CORE, COLS], _F16, kind="ExternalInput")
    g = nc.dram_tensor("g", [128, 128 + P_S], _F16, kind="ExternalInput")
    y = nc.dram_tensor("y", [ROWS_PER_CORE, COLS], _F16, kind="ExternalOutput")

    S = P_S
    gb = _Greedy(nc)
    # ACT leads: its first HWDGE DMA issues without the cross-ring
    # serializing wait the sem-assignment pass pins on the other ring's
    # opener, so gt (needed by the first conv matmul) lands earliest
    spa = [nc.scalar, nc.sync]
    tp_i = 0

    with TileContext(nc) as tc:
        with (
            tc.tile_pool(name="cpool", bufs=1) as cpool,
            tc.tile_pool(name="upool", bufs=P_U) as upool,
            tc.tile_pool(name="pwpool", bufs=1, space="PSUM") as pwpool,
            tc.tile_pool(name="popool", bufs=P_PO, space="PSUM") as popool,
            tc.tile_pool(name="ypool", bufs=P_Y) as ypool,
        ):
            # PE warm-up: zero matmuls into a scratch PSUM bank start the
            # p-state ramp clock while the first transposed loads are in
            # flight (their ~2.6us DGE+sem latency would otherwise be spent
            # ramping through the slow p-states on real work)
            zt = cpool.tile([128, 512], _F16, tag="z")
            nc.vector.memset(zt[:], 0.0)
            pw = pwpool.tile([128, 512], _F32, tag="pw")
            for _ in range(N_WARM):
                nc.tensor.matmul(
                    pw[:], lhsT=zt[:, 0:128], rhs=zt[:], start=True, stop=True
                )

            gt = cpool.tile([128, 128 + S], _F16, tag="g")
            nc.scalar.dma_start(gt[:], g[:])
            gb.busy["ACT"] += 500.0

            for rc in range(NRC):
                rs = slice(128 * rc, 128 * (rc + 1))

                # 2 transposed block loads per rc, issued up front:
                # ub[t, 128j + r] = x[rs, 2048*blk + 128j + t][r]
                u_blocks = {}
                for blk in range(NBLK):
                    ut = upool.tile([128, 2048], _F16, tag=f"u{blk}")
                    eng = spa[tp_i % 2]
                    tp_i += 1
                    eng.dma_start_transpose(
                        ut[:].rearrange("p (j r) -> p j r", j=16),
                        x[rs, 2048 * blk : 2048 * (blk + 1)],
                    )
                    gb.busy["SP" if eng is nc.sync else "ACT"] += _C_TPOSE
                    u_blocks[blk] = ut

                def u_tile(b, u_blocks=u_blocks):
                    # time tile b (0..31) -> 128-col slice of its block
                    return u_blocks[b // 16][:, 128 * (b % 16) : 128 * (b % 16 + 1)]

                for grp in range(4):  # 1024-col conv groups (2-bank PSUM)
                    pt = popool.tile([128, 1024], _F32, tag="po")
                    for h in range(2):  # 512-col halves, one PSUM bank each
                        q = 2 * grp + h
                        plan = []
                        for j in range(4):
                            plan.append((128 * j, 128, 4 * q + j, 0))
                        for j in range(4):
                            if 4 * q + j - 1 >= 0:
                                plan.append((128 * j, S, 4 * q + j - 1, 128))
                        for i, (col, w, b, goff) in enumerate(plan):
                            nc.tensor.matmul(
                                pt[:, 512 * h + col : 512 * h + col + w],
                                lhsT=u_tile(b),
                                rhs=gt[:, goff : goff + w],
                                start=(i == 0),
                                stop=(i == len(plan) - 1),
                            )

                    # y staging: 2048-col fp16 groups, cast-copied in
                    # 1024-col units split DVE/ACT (Pool's gpsimd cannot
                    # read PSUM on hardware)
                    q = 2 * grp + 1
                    if grp % 2 == 0:
                        yt = ypool.tile([128, 2048], _F16, tag="y")
                    _, eng = gb.pick(("DVE", "ACT"), _C_COPY)
                    cfn = eng.copy if eng is nc.scalar else eng.tensor_copy
                    cfn(yt[:, 1024 * (grp % 2) : 1024 * (grp % 2 + 1)], pt[:])
                    if q % 4 == 3:
                        c0 = 2048 * (q // 4)
                        # store schedule: early stores stay on the SP/ACT
                        # HWDGE ring (densifying it keeps the sem-merge
                        # pass's lane waits pointing at nearby DMAs instead
                        # of the final transpose); Pool takes rc2, and the
                        # tail is split 1024-wide across all three engines
                        if rc == NRC - 1:
                            for hh in range(2):
                                _, eng = gb.pick(("SP", "ACT", "Pool"), _C_STORE / 2)
                                eng.dma_start(
                                    y[rs, c0 + 1024 * hh : c0 + 1024 * (hh + 1)],
                                    yt[:, 1024 * hh : 1024 * (hh + 1)],
                                )
                        elif rc == 2:
                            gb.busy["Pool"] += _C_STORE
                            nc.gpsimd.dma_start(y[rs, c0 : c0 + 2048], yt[:])
                        else:
                            _, eng = gb.pick(("SP", "ACT"), _C_STORE)
                            eng.dma_start(y[rs, c0 : c0 + 2048], yt[:])
    if legalize:
        _legalize_waits(nc)
    return nc


def _legalize_waits(nc: bass.Bass) -> None:
    """This toolchain's walrus accepts at most ONE semaphore wait per
    instruction (Drain/EventSemaphore excepted), but Tile's semaphore
    assignment freely emits 2-3. Hoist extra waits onto injected same-engine
    NoOps placed immediately before the instruction — engines execute their
    stream serially (and a DMA trigger precedes its descriptor execution),
    so waiting earlier on the same engine preserves semantics.
    """
    for fn in nc.m.functions:
        for blk in fn.blocks:
            out = []
            changed = False
            for i in blk.instructions:
                tn = type(i).__name__
                si = i.sync_info
                cap = 2 if tn == "InstEventSemaphore" else 1
                if si is not None and len(si.on_wait) > cap:
                    waits = list(si.on_wait)
                    for w in waits[:-cap]:
                        out.append(
                            mybir.InstNoOp(
                                name=nc.get_next_instruction_name(),
                                ins=[],
                                outs=[],
                                engine=i.engine,
                                sync_info=mybir.SyncInfo(
                                    on_wait=[w], on_update=[]
                                ),
                            )
                        )
                    i.sync_info = mybir.SyncInfo(
                        on_wait=waits[-cap:], on_update=list(si.on_update)
                    )
                    changed = True
                out.append(i)
            if changed:
                blk.instructions = out


_PROGRAM = None


def kernel(**inputs: np.ndarray) -> np.ndarray:
    global _PROGRAM
    x = np.asarray(inputs["inputs"], dtype=np.float32)
    h = np.asarray(inputs["kernel"], dtype=np.float32)[0]
    assert x.shape == (ROWS, COLS) and h.shape == (K_TAPS,)

    g_cat = _build_g_cat(h)
    xm = x.astype(np.float16)
    xm[:, :K_TAPS] = 0

    if _PROGRAM is None:
        _PROGRAM = _build_program()

    in_maps = [
        {
            "x": xm[ROWS_PER_CORE * c : ROWS_PER_CORE * (c + 1)],
            "g": g_cat,
        }
        for c in range(N_CORES)
    ]
    # the axon-proxied device occasionally reports a transient
    # NRT_EXEC_UNIT_UNRECOVERABLE; a retry succeeds
    last_err = None
    for _ in range(3):
        try:
            res = run_bass_kernel_spmd(
                _PROGRAM, in_maps, list(range(N_CORES))
            ).results
            break
        except Exception as e:  # noqa: BLE001
            last_err = e
            time.sleep(2.0)
    else:
        raise last_err
    out = np.concatenate([res[c]["y"] for c in range(N_CORES)], axis=0)
    return out.astype(np.float32)
